# revision 21
# baseline (speedup 1.0000x reference)
"""Multi-head attention on 8 TRN2 NeuronCores.

Reference computation (per batch b):
  q = x @ w_q;  k, v = split(x @ w_kv);  per head: softmax(q k^T / 8) v
  out = ctx @ w_out + b_out

Sharding: core c handles batch b = c // 2 and head-half hh = c % 2
(8 of 16 heads). Per-core work is a perfectly balanced 1/8 of total
MACs. Each core computes a partial out^T (its 8 heads' contribution,
transposed); the host sums the two partials per batch, adds the bias
and transposes back.

The kernel is one software-pipelined stream of 16 "phases" (head-pair
p x q-chunk c, 512 q positions each) of 16 k-tile slots. Per slot:

  scores^T [128 sk, 1024] = two row-tiled K=64 matmuls (both heads
    concurrently in PE rows 0-63 / 64-127)
  P^T = Exp(scale * scores^T) on ScalarE -> fp16 SBUF tile. ScalarE is
    the critical engine (~1.07us per slot); everything else is
    scheduled around keeping it fed.
  ctx^T accumulation runs ONE FULL PHASE BEHIND (P tiles persist 16
    slots), so psum needs only: 2 score bufs + 2 aux bufs + 1 ctx buf
    = 8 banks, and the chunk-end denominator/copy latency never stalls
    the PE queue.
  aux: one 4-matmul slice of a projection group (x @ w_q/w_k/w_v, or
    the out-projection) is interleaved per slot from a deadline-tagged
    queue: V tiles before the lagged ctx needs them, q/k of pair p+1
    before its phases, out-projection once all four ct q-slices of a
    q-column are normalized.

Everything is fp16 (same PE/DVE rate as bf16, 8x the mantissa), so the
softmax denominator (ones columns riding in the V matmul -> psum rows
64:67), the P tiles, and all projections stay well inside the 2e-2
error budget. Host-side sum of the two per-batch partials in fp32.
"""

import numpy as np

import concourse.bacc as bacc
import concourse.tile as tile
import concourse.mybir as mybir
from concourse.bass_utils import run_bass_kernel_spmd

FP32 = mybir.dt.float32
FP16 = mybir.dt.float16
EXP = mybir.ActivationFunctionType.Exp

B, S, DL = 4, 2048, 1024
H, DH = 16, 64          # global heads
NH = 8                  # heads per core
HD = NH * DH            # 512 feature cols per core
NPAIR = NH // 2         # 4 head pairs
KT = DL // 128          # 8 k-tiles over d_latent
ST = S // 128           # 16 tiles over sequence
VW = DH + 4             # V cols + ones cols (denominator ride-along)
SCALE = 1.0 / np.sqrt(DH)

N_CORES = 8
NPH = NPAIR * 4         # 16 phases: (pair, q-chunk)


def _build(reps: int = 1, loop: int = 0, ablate=()):
    nc = bacc.Bacc(None, target_bir_lowering=False)

    xT = nc.dram_tensor("xT", [DL, S], FP16, kind="ExternalInput")
    wq = nc.dram_tensor("wq", [DL, HD], FP16, kind="ExternalInput")
    wk = nc.dram_tensor("wk", [DL, HD], FP16, kind="ExternalInput")
    wv = nc.dram_tensor("wv", [DL, HD], FP16, kind="ExternalInput")
    wo = nc.dram_tensor("wo", [HD, DL], FP16, kind="ExternalInput")
    out = nc.dram_tensor("out", [DL, S], FP16, kind="ExternalOutput")

    with tile.TileContext(nc) as tc:
        with (
            tc.tile_pool(name="persist", bufs=1) as pp,
            tc.tile_pool(name="pt", bufs=20) as ptp,
            tc.tile_pool(name="small", bufs=3) as smp,
            tc.tile_pool(name="rd", bufs=2) as rdp,
            tc.tile_pool(name="outsb", bufs=2) as osp,
            tc.tile_pool(name="psA", bufs=2, space="PSUM") as psA,
            tc.tile_pool(name="psX", bufs=2, space="PSUM") as psX,
            tc.tile_pool(name="psB", bufs=1, space="PSUM") as psB,
        ):
            pools = (pp, ptp, smp, rdp, osp, psA, psX, psB)
            if loop:
                with tc.For_i(0, loop, 1):
                    _body(nc, tc, pools, xT, wq, wk, wv, wo, out, ablate)
            else:
                for _ in range(reps):
                    _body(nc, tc, pools, xT, wq, wk, wv, wo, out, ablate)
    nc.compile()
    return nc


def _body(nc, tc, pools, xT, wq, wk, wv, wo, out, ablate=()):
    pp, ptp, smp, rdp, osp, psA, psX, psB = pools

    # ---- persistent tiles (tag-keyed; reused across reps) ----
    xt = [pp.tile([128, S], FP16, tag=f"xt{k}", name=f"xt{k}") for k in range(KT)]
    wq_sb = [pp.tile([128, HD], FP16, tag=f"wq{k}", name=f"wq{k}") for k in range(KT)]
    wk_sb = [pp.tile([128, HD], FP16, tag=f"wk{k}", name=f"wk{k}") for k in range(KT)]
    wv_sb = [pp.tile([128, HD], FP16, tag=f"wv{k}", name=f"wv{k}") for k in range(KT)]
    wo_sb = [pp.tile([128, DL], FP16, tag=f"wo{t}", name=f"wo{t}") for t in range(NPAIR)]
    qt = [pp.tile([128, S], FP16, tag=f"qt{p}", name=f"qt{p}") for p in range(NPAIR)]
    kt_ = [pp.tile([128, S], FP16, tag=f"kt{p}", name=f"kt{p}") for p in range(NPAIR)]
    vt = [pp.tile([128, NH * VW], FP16, tag=f"vt{m}", name=f"vt{m}") for m in range(ST)]
    ct = [pp.tile([128, S], FP16, tag=f"ct{t}", name=f"ct{t}") for t in range(NPAIR)]
    # out-projection partial sums for the last q-column (t = 0..2 terms)
    po = [pp.tile([128, 512], FP16, tag=f"po{mt}", name=f"po{mt}") for mt in range(KT)]

    # ---- input DMAs ----
    for k in range(KT):
        nc.sync.dma_start(xt[k][:, :], xT[k * 128:(k + 1) * 128, :])
        nc.sync.dma_start(wk_sb[k][:, :], wk[k * 128:(k + 1) * 128, :])
        nc.sync.dma_start(wq_sb[k][:, :], wq[k * 128:(k + 1) * 128, :])
        nc.sync.dma_start(wv_sb[k][:, :], wv[k * 128:(k + 1) * 128, :])
    for t in range(NPAIR):
        nc.sync.dma_start(wo_sb[t][:, :], wo[t * 128:(t + 1) * 128, :])
    if 'proj' in ablate:
        for p_ in range(NPAIR):
            nc.sync.dma_start(qt[p_][:, :], xT[p_ * 128:(p_ + 1) * 128, :])
            nc.sync.dma_start(kt_[p_][:, :], xT[p_ * 128:(p_ + 1) * 128, :])
    if 'vproj' in ablate:
        for m_ in range(ST):
            nc.vector.memset(vt[m_][:, :], 0.01)
    if 'attn' in ablate:
        for t_ in range(NPAIR):
            nc.sync.dma_start(ct[t_][:, :], xT[t_ * 128:(t_ + 1) * 128, :])

    # ================= aux slice machinery =================
    # aux_q holds GROUPS [deadline_end, earliest, [slice_fns]]. Slices
    # only ever pop from the HEAD group, so one group's psX tiles are
    # fully written+copied before the next group allocates — required
    # for the paired groups below, which hold both psX bufs at once.
    aux_q = []

    def qk_group_pair(dst, w_sb, p_, qqA, qqB, deadline):
        """x @ w -> dst for TWO q-columns jointly, k-major: consecutive
        matmuls share the same stationary w-slice (weight reload is the
        dominant per-matmul cost: HW measured 418 vs 141 ns/MM)."""
        cell = {}
        rA = slice(qqA * 512, (qqA + 1) * 512)
        rB = slice(qqB * 512, (qqB + 1) * 512)

        def emit(lo, hi):
            if 'a' not in cell:
                cell['a'] = psX.tile([128, 512], FP32, tag="aux", name="aux")
                cell['b'] = psX.tile([128, 512], FP32, tag="aux", name="aux")
            for k in range(lo, hi):
                w_ap = w_sb[k][:, p_ * 128:(p_ + 1) * 128]
                nc.tensor.matmul(cell['a'][:, :], w_ap, xt[k][:, rA],
                                 start=(k == 0), stop=(k == KT - 1))
                nc.tensor.matmul(cell['b'][:, :], w_ap, xt[k][:, rB],
                                 start=(k == 0), stop=(k == KT - 1))
            if hi == KT:
                nc.vector.tensor_copy(dst[:, rA], cell['a'][:, :])
                nc.vector.tensor_copy(dst[:, rB], cell['b'][:, :])
        aux_q.append([deadline, -99,
                      [(lambda lo: lambda: emit(lo, lo + 2))(2 * s)
                       for s in range(4)]])

    def out_group_pair(mt, qqA, qqB, deadline, termsB=(0, 1, 2, 3),
                       partialB=False):
        """out-projection for TWO q-columns jointly, t-major: consecutive
        matmuls share the same stationary wo-slice."""
        msl = slice(mt * 128, (mt + 1) * 128)
        rA = slice(qqA * 512, (qqA + 1) * 512)
        rB = slice(qqB * 512, (qqB + 1) * 512)
        tsB = list(termsB)

        def emit():
            psa = psX.tile([128, 512], FP32, tag="aux", name="aux")
            psb = psX.tile([128, 512], FP32, tag="aux", name="aux")
            for t in range(NPAIR):
                w_ap = wo_sb[t][:, msl]
                nc.tensor.matmul(psa[:, :], w_ap, ct[t][:, rA],
                                 start=(t == 0), stop=(t == NPAIR - 1))
                if t in tsB:
                    jj = tsB.index(t)
                    nc.tensor.matmul(psb[:, :], w_ap, ct[t][:, rB],
                                     start=(jj == 0), stop=(jj == len(tsB) - 1))
            ob = osp.tile([128, 512], FP16, tag="ob", name="ob")
            nc.vector.tensor_copy(ob[:, :], psa[:, :])
            nc.sync.dma_start(out[msl, rA], ob[:, :])
            if partialB:
                nc.vector.tensor_copy(po[mt][:, :], psb[:, :])
            else:
                ob2 = osp.tile([128, 512], FP16, tag="ob", name="ob")
                nc.vector.tensor_copy(ob2[:, :], psb[:, :])
                nc.sync.dma_start(out[msl, rB], ob2[:, :])
        aux_q.append([deadline, deadline, [emit]])

    def v_group(m, deadline):
        """x-tile @ w_v -> vt[m] fp16 (+ ones cols), 2 slices."""
        cell = {}

        def emit(lo, hi):
            if 'tile' not in cell:
                cell['tile'] = psX.tile([128, 512], FP32, tag="aux", name="aux")
            ps = cell['tile']
            for k in range(lo, hi):
                nc.tensor.matmul(ps[:, :],
                                 xt[k][:, m * 128:(m + 1) * 128],
                                 wv_sb[k][:, :],
                                 start=(k == 0), stop=(k == KT - 1))
            if hi == KT:
                v3 = vt[m][:, :].rearrange("p (h c) -> p h c", c=VW)
                nc.vector.tensor_copy(
                    v3[:, :, 0:DH],
                    ps[:, :].rearrange("p (h c) -> p h c", h=NH))
                nc.vector.memset(v3[:, :, DH:VW], 1.0)
        aux_q.append([deadline, -99,
                      [lambda: emit(0, 4), lambda: emit(4, KT)]])

    def out_group(mt, qq, deadline, terms=range(NPAIR), partial_from=None):
        """sum_t wo[t]^T @ ct[t] for one (m-tile, q-512-slice)."""
        msl = slice(mt * 128, (mt + 1) * 128)
        rsl = slice(qq * 512, (qq + 1) * 512)
        terms = list(terms)

        def emit():
            ps = psX.tile([128, 512], FP32, tag="aux", name="aux")
            for ii, t in enumerate(terms):
                nc.tensor.matmul(ps[:, :],
                                 wo_sb[t][:, msl],
                                 ct[t][:, rsl],
                                 start=(ii == 0), stop=(ii == len(terms) - 1))
            if partial_from is None and len(terms) == NPAIR:
                ob = osp.tile([128, 512], FP16, tag="ob", name="ob")
                nc.vector.tensor_copy(ob[:, :], ps[:, :])
                nc.sync.dma_start(out[msl, rsl], ob[:, :])
            elif partial_from is None:            # partial: store to SBUF
                nc.vector.tensor_copy(po[mt][:, :], ps[:, :])
            else:                                  # final: add stored partial
                ob = osp.tile([128, 512], FP16, tag="ob", name="ob")
                nc.vector.tensor_add(ob[:, :], ps[:, :], partial_from[:, :])
                nc.sync.dma_start(out[msl, rsl], ob[:, :])
        aux_q.append([deadline, deadline, [emit]])

    # ---- build the aux queue (deadline order) ----
    PH_PRE = sorted(((p, c) for p in range(NPAIR) for c in range(4)),
                    key=lambda pc: (pc[0] + pc[1], pc[0]))
    IDX_PRE = {pc: i for i, pc in enumerate(PH_PRE)}
    first_ph = {p_: min(IDX_PRE[(p_, c_)] for c_ in range(4))
                for p_ in range(NPAIR)}
    if 'proj' not in ablate:
        for p_ in range(NPAIR):
            fp_ = first_ph[p_]
            for qqA, qqB in ((0, 1), (2, 3)):
                # deadline = the EARLIER consumer of the two q-columns
                kd = min((16 * fp_ + 4 * qq - 2 if fp_ else
                          (-2 if qq == 0 else 4 * qq - 2))
                         for qq in (qqA, qqB))
                qd = min((-2 if (p_ == 0 and qq == 0) else
                          16 * IDX_PRE[(p_, qq)] - 2) for qq in (qqA, qqB))
                qk_group_pair(kt_[p_], wk_sb, p_, qqA, qqB, kd)
                qk_group_pair(qt[p_], wq_sb, p_, qqA, qqB, qd)
    if 'vproj' not in ablate:
        for m in range(ST):
            v_group(m, 14 + m)                 # needed at slot 16+m (lagged ctx)
    if 'out' not in ablate:
        # q-column qq is final after flush_norm of its LAST phase j*
        # (slot 16*(j*+2)+6). Pair (qq0,qq1) and (qq2,qq3-partial) so
        # consecutive matmuls share each wo-slice. Column 3 completes
        # past the end -> its last pair's term runs post-loop.
        ready = {}
        for qq in range(4):
            jstar = max(IDX_PRE[(p_, qq)] for p_ in range(NPAIR))
            ready[qq] = 16 * (jstar + 2) + 7
        jstar3 = max(IDX_PRE[(p_, 3)] for p_ in range(NPAIR))
        jpen3 = sorted(IDX_PRE[(p_, 3)] for p_ in range(NPAIR))[-2]
        pterms = [p_ for p_ in range(NPAIR) if IDX_PRE[(p_, 3)] != jstar3]
        pstar = [p_ for p_ in range(NPAIR) if IDX_PRE[(p_, 3)] == jstar3]
        r01 = max(ready[0], ready[1])
        r23 = max(ready[2], 16 * (jpen3 + 2) + 7)
        for mt in range(KT):
            out_group_pair(mt, 0, 1, r01 + mt)
            out_group_pair(mt, 2, 3, r23 + mt, termsB=pterms, partialB=True)
        cell_final = (3, pstar)
    aux_q.sort(key=lambda it: it[0])

    # ================= the main pipelined stream =================
    PH = sorted(((p, c) for p in range(NPAIR) for c in range(4)),
                key=lambda pc: (pc[0] + pc[1], pc[0]))
    IDX = {pc: i for i, pc in enumerate(PH)}
    phase_pt = [[None] * ST for _ in range(NPH)]
    phase_ctx = [None] * NPH
    phase_rsrc = [None] * NPH
    norm_pend = []
    BCAST_MASK = [0] * 32
    do_attn = 'attn' not in ablate
    do_ctx = do_attn and 'ctx' not in ablate

    def flush_norm():
        while norm_pend:
            j_ = norm_pend.pop(0)
            pj_, cj_ = PH[j_]
            qsl_ = slice(cj_ * 512, (cj_ + 1) * 512)
            rdst = rdp.tile([128, 1024], FP16, tag="rdst", name="rdst")
            nc.vector.stream_shuffle(rdst[:, :], phase_rsrc[j_][:, :], BCAST_MASK)
            for hi_ in range(2):
                psl = slice(hi_ * 64, (hi_ + 1) * 64)
                csl = ct[pj_][psl, qsl_]
                nc.vector.tensor_mul(csl, csl,
                                     rdst[psl, hi_ * 512:hi_ * 512 + 512])

    def emit_aux(g, default_one=True):
        emitted = 0
        while aux_q:
            d_end, earliest, slices = aux_q[0]
            due = (d_end - len(slices) + 1) <= g
            fill = default_one and emitted == 0 and earliest <= g
            if not (due or fill):
                break
            slices.pop(0)()
            emitted += 1
            if not slices:
                aux_q.pop(0)

    emit_aux(-1, default_one=False)   # prologue groups (deadline < 0)

    # scores are emitted ONE SLOT AHEAD of their exp, so ScalarE never
    # waits on the current slot's matmuls (its input landed a full slot
    # ago and the psum rotation has a slot of slack).
    sc_pend = {}

    def emit_scores(gs):
        if gs >= 16 * NPH or not do_attn:
            return
        i_, ki_ = divmod(gs, 16)
        p_, c_ = PH[i_]
        qsl_ = slice(c_ * 512, (c_ + 1) * 512)
        ksl_ = slice(ki_ * 128, (ki_ + 1) * 128)
        sc = psA.tile([128, 1024], FP32, tag="sc", name="sc")
        nc.tensor.matmul(sc[:, 0:512], kt_[p_][0:64, ksl_],
                         qt[p_][0:64, qsl_], start=True, stop=True,
                         tile_position=(0, 0))
        nc.tensor.matmul(sc[:, 512:1024], kt_[p_][64:128, ksl_],
                         qt[p_][64:128, qsl_], start=True, stop=True,
                         tile_position=(64, 0))
        sc_pend[gs] = sc

    emit_scores(0)

    TAIL = 24
    for g in range(16 * NPH + TAIL):
        i, ki = divmod(g, 16)
        # ---- exp for phase i (scores already in psum) ----
        if i < NPH and do_attn:
            sc = sc_pend.pop(g)
            pt = ptp.tile([128, 1024], FP16, tag="pt", name="pt")
            nc.scalar.activation(pt[:, :], sc[:, :], EXP, scale=SCALE)
            phase_pt[i][ki] = pt
        # ---- lagged ctx for phase j = i-1 ----
        j = i - 1
        if 0 <= j < NPH and do_ctx:
            pj, cj = PH[j]
            qslj = slice(cj * 512, (cj + 1) * 512)
            if ki == 0:
                phase_ctx[j] = psB.tile([VW, 1024], FP32, tag="ctx", name="ctxp")
                rs = smp.tile([128, 1024], FP16, tag="rsrc", name="rsrc")
                nc.vector.memset(rs[:, :], 1.0)
                phase_rsrc[j] = rs
            ctxp = phase_ctx[j]
            ptj = phase_pt[j][ki]
            for hi in range(2):
                nc.tensor.matmul(ctxp[:, hi * 512:(hi + 1) * 512],
                                 vt[ki][:, (2 * pj + hi) * VW:(2 * pj + hi + 1) * VW],
                                 ptj[:, hi * 512:(hi + 1) * 512],
                                 start=(ki == 0), stop=(ki == ST - 1))
            phase_pt[j][ki] = None
            if ki == ST - 1 and 'norm' not in ablate:
                rs = phase_rsrc[j]
                for hi in range(2):
                    with nc.allow_low_precision(reason="softmax denom recip"):
                        nc.vector.reciprocal(rs[0:4, hi * 512:(hi + 1) * 512],
                                             ctxp[DH:DH + 4, hi * 512:(hi + 1) * 512])
                    nc.vector.tensor_copy(ct[pj][hi * 64:(hi + 1) * 64, qslj],
                                          ctxp[0:DH, hi * 512:(hi + 1) * 512])
                for q_ in (32, 64, 96):
                    nc.vector.tensor_copy(rs[q_:q_ + 4, :], rs[0:4, :])
                if 'chain' not in ablate:
                    norm_pend.append(j)
        if ki == 6:
            flush_norm()
        emit_aux(g)
        emit_scores(g + 1)

    flush_norm()
    # final out-projection terms for the last-finishing q-column
    if 'out' not in ablate:
        qq_f, pstar_f = cell_final
        for mt in range(KT):
            out_group(mt, qq_f, 0, terms=pstar_f, partial_from=po[mt])
        for _, _, slices in aux_q:
            for fn in slices:
                fn()
        aux_q.clear()


_NC_CACHE = {}


def _get_nc(reps: int = 1):
    if reps not in _NC_CACHE:
        _NC_CACHE[reps] = _build(reps)
    return _NC_CACHE[reps]


def shard_inputs(x, w_q, w_kv, w_out):
    """Full inputs -> per-core in_maps (host-side layout prep)."""
    ins = []
    for c in range(N_CORES):
        b, hh = c // 2, c % 2
        fsl = slice(hh * HD, (hh + 1) * HD)
        ins.append({
            "xT": np.ascontiguousarray(x[b].T).astype(np.float16),
            "wq": np.ascontiguousarray(w_q[:, fsl]).astype(np.float16),
            "wk": np.ascontiguousarray(w_kv[:, fsl]).astype(np.float16),
            "wv": np.ascontiguousarray(w_kv[:, H * DH:][:, fsl]).astype(np.float16),
            "wo": np.ascontiguousarray(w_out[fsl, :]).astype(np.float16),
        })
    return ins


def unshard_output(results, b_out):
    out = np.empty((B, S, DL), np.float32)
    for b in range(B):
        acc = (results[2 * b]["out"].astype(np.float32)
               + results[2 * b + 1]["out"].astype(np.float32))   # [DL, S]
        out[b] = acc.T + b_out
    return out


def kernel(x, w_q, w_kv, w_out, b_out):
    nc = _get_nc()
    ins = shard_inputs(x, w_q, w_kv, w_out)
    res = run_bass_kernel_spmd(nc, ins, core_ids=list(range(N_CORES)))
    return unshard_output(res.results, b_out)
